# revision 9
# baseline (speedup 1.0000x reference)
"""Trainium2 Bass kernel: conv2d(3x3, VALID) + bias -> channel-min -> tanh(tanh).

Full inputs in, full output out. Data-parallel over batch across 8 NeuronCores.

Scheme (fp8 DoubleRow weight-stationary conv + PE-transpose channel-min):
  - Conv as matmul, weight-stationary: M packs (delta, oc) = 128 output
    partitions (delta = output-row parity via the khe = delta + kh trick),
    contraction K = (khe in [0,4), ic) = 64 packed as fp8 DoubleRow
    [32 partitions x 2 k-tiles], moving tensor = 4 row-pairs x 128 cols of
    the khe-shifted fp8 image (N = 512, kw enters as a free-dim offset).
    DoubleRow streams N=512 in ~256 PE cycles - 2x over bf16.
  - Two images per pair run interleaved on PE row ranges 0-31 / 32-63
    (tile_position row tiling) so stationary loads hide under streams.
  - PSUM f32 [128=(delta,oc), 512] is evacuated to SBUF bf16 with the conv
    bias fused (ScalarE Identity+bias, VectorE tensor_scalar for a share).
  - The channel-min needs (delta,oc) moved off partitions: each 128x128
    t-row tile is TRANSPOSED either by the PE (matmul is_transpose, bf16 ->
    bf16 PSUM) or by the DMA xbar (SBUF->SBUF) - split between the two to
    balance engine load. Then a single 16-bit DVE tensor_reduce per group
    computes min over oc keeping (t, delta).
  - Double tanh on ScalarE; contiguous store as [img, w', h']; host
    transposes back.
"""

import os
import sys

for _p in ("/opt/trn_rl_repo", "/root/.axon_site/_ro/trn_rl_repo"):
    if os.path.isdir(_p) and _p not in sys.path:
        sys.path.insert(0, _p)

import numpy as np
import ml_dtypes

import concourse.bass as bass
import concourse.bacc as bacc
import concourse.tile as tile
from concourse import mybir
from concourse.bass_utils import run_bass_kernel_spmd

N_CORES = 8
B, IC, H, W = 128, 16, 128, 128
OC, KSZ = 64, 3
HO, WO = H - KSZ + 1, W - KSZ + 1  # 126, 126
B_LOC = B // N_CORES  # 16
PAIRS = B_LOC // 2  # 8
FLAT = H * W  # 16384
T = HO // 2  # 63 row-pairs per image (h' = 2t + delta)

BF16 = mybir.dt.bfloat16
FP8 = mybir.dt.float8e4
F32 = mybir.dt.float32
MIN = mybir.AluOpType.min
DR = mybir.MatmulPerfMode.DoubleRow

# t-groups of up to 4 row-pairs -> matmul N = cnt*128
GROUPS = [(t0, min(4, T - t0)) for t0 in range(0, T, 4)]  # 16 groups, last cnt=3


def _build_program():
    nc = bacc.Bacc(None)
    xr_hbm = nc.declare_dram_parameter(
        "xrep", [PAIRS, 64, 2 * FLAT], FP8, isOutput=False
    )
    w_hbm = nc.declare_dram_parameter("wts", [64, 2 * 384], FP8, isOutput=False)
    b_hbm = nc.declare_dram_parameter("bias", [128, 1], F32, isOutput=False)
    i_hbm = nc.declare_dram_parameter("iden", [128, 128], BF16, isOutput=False)
    y_hbm = nc.declare_dram_parameter("y", [B_LOC, WO, HO], F32, isOutput=True)

    with tile.TileContext(nc) as tc:
        with (
            tc.tile_pool(name="const", bufs=1) as const,
            tc.tile_pool(name="xrp", bufs=2) as xrp,
            tc.tile_pool(name="psum", bufs=4, space="PSUM") as psump,
            tc.tile_pool(name="tpps", bufs=4, space="PSUM") as tppsp,
            tc.tile_pool(name="evac", bufs=6) as evacp,
            tc.tile_pool(name="tpsb", bufs=6) as tpsbp,
            tc.tile_pool(name="fin", bufs=4) as finp,
            tc.tile_pool(name="outp", bufs=4) as outp,
        ):
            w_sb = const.tile([128, 2 * 384], FP8)
            b_sb = const.tile([128, 1], F32)
            iden = const.tile([128, 128], BF16)
            nc.sync.dma_start(w_sb[0:64, :], w_hbm[:])
            nc.sync.dma_start(b_sb[:], b_hbm[:])
            nc.sync.dma_start(iden[:], i_hbm[:])
            wv = w_sb.rearrange("p (k c) -> p k c", k=2)  # [64, 2, 384]

            xr_tiles = {}

            def load_pair(p):
                xr_t = xrp.tile([128, 2 * FLAT], FP8, name="xr", tag="xr")
                nc.sync.dma_start(xr_t[0:64, :], xr_hbm[p])
                xr_tiles[p] = xr_t

            load_pair(0)
            ucount = 0
            for pair in range(PAIRS):
                if pair + 1 < PAIRS:
                    load_pair(pair + 1)
                xr = xr_tiles.pop(pair)
                # [64, 2 kt, 64 row-pairs, 256]
                xrv = xr.rearrange("p (k r q) -> p k r q", k=2, q=2 * W)

                fins = [
                    finp.tile([128, HO], BF16, tag="fin", name="fin")
                    for _ in range(2)
                ]

                for g, (t0, cnt) in enumerate(GROUPS):
                    n = cnt * 128
                    ps2 = [
                        psump.tile([128, 512], F32, name="ps", tag="ps")
                        for _ in range(2)
                    ]
                    for kw in range(3):
                        for q in range(2):
                            nc.tensor.matmul(
                                ps2[q][:, :n],
                                wv[32 * q : 32 * q + 32, :, kw * 128 : kw * 128 + 128],
                                xrv[
                                    32 * q : 32 * q + 32,
                                    :,
                                    t0 : t0 + cnt,
                                    kw : kw + 128,
                                ],
                                start=(kw == 0),
                                stop=(kw == 2),
                                tile_position=(32 * q, 0),
                                perf_mode=DR,
                                skip_group_check=True,
                            )
                    for q in range(2):
                        # evacuate PSUM -> SBUF bf16 with fused bias
                        ev = evacp.tile([128, 512], BF16, tag="ev", name="ev")
                        ucount += 1
                        if ucount % 3 == 0:
                            nc.vector.tensor_scalar(
                                ev[:, :n], ps2[q][:, :n], b_sb[:, 0:1], None,
                                mybir.AluOpType.add,
                            )
                        else:
                            nc.scalar.activation(
                                ev[:, :n], ps2[q][:, :n],
                                mybir.ActivationFunctionType.Identity,
                                bias=b_sb[:, 0:1],
                            )
                        # transpose the cnt t-row tiles: alternate PE / DMA
                        if g % 2 == 0:
                            tp = tppsp.tile([128, 512], BF16, name="tpp", tag="tpp")
                            for j in range(cnt):
                                nc.tensor.transpose(
                                    tp[:, j * 128 : (j + 1) * 128],
                                    ev[:, j * 128 : (j + 1) * 128],
                                    iden[:],
                                )
                        else:
                            tp = tpsbp.tile([128, 512], BF16, name="tps", tag="tps")
                            for j in range(cnt):
                                nc.sync.dma_start_transpose(
                                    tp[:, j * 128 : (j + 1) * 128],
                                    ev[:, j * 128 : (j + 1) * 128],
                                )
                        # min over oc keeping (t, delta):
                        # tp layout [128=w, (t, delta, oc)]
                        tpv = tp.rearrange("p (j d c) -> p j d c", d=2, c=64)
                        nc.vector.tensor_reduce(
                            fins[q][:, 2 * t0 : 2 * t0 + 2 * cnt],
                            tpv[:, :cnt, :, :],
                            axis=mybir.AxisListType.X,
                            op=MIN,
                        )

                # double tanh + store  (partitions = w'; 126..127 garbage)
                for q in range(2):
                    th = outp.tile([128, HO], F32, tag="th", name="th")
                    ot = outp.tile([128, HO], F32, tag="ot", name="ot")
                    nc.scalar.activation(
                        th[0:WO, :], fins[q][0:WO, :],
                        mybir.ActivationFunctionType.Tanh,
                    )
                    nc.scalar.activation(
                        ot[0:WO, :], th[0:WO, :],
                        mybir.ActivationFunctionType.Tanh,
                    )
                    nc.scalar.dma_start(y_hbm[pair * 2 + q], ot[0:WO, :])
    nc.finalize()
    return nc


_NC_CACHE = None


def _get_program():
    global _NC_CACHE
    if _NC_CACHE is None:
        _NC_CACHE = _build_program()
    return _NC_CACHE


def _host_prep(x, conv_weight, conv_bias):
    # khe-shifted fp8 copies: xr8[b, c=(khe,ic), r, w] = x[b, ic, r+khe, w]
    x8 = x.astype(ml_dtypes.float8_e4m3)
    xr = np.zeros((B, 4 * IC, H, W), dtype=ml_dtypes.float8_e4m3)
    for khe in range(4):
        xr[:, khe * IC : (khe + 1) * IC, : H - khe, :] = x8[:, :, khe:, :]
    # DoubleRow packing: partition p holds contraction rows c=p (kt0) and
    # c=32+p (kt1) contiguously
    xr = xr.reshape(B, 2, 32, FLAT).transpose(0, 2, 1, 3)  # [B, 32, 2, FLAT]

    # weights: wl[p, kt, kw*128 + delta*64 + oc] = w[oc, ic, khe-delta, kw]
    wl = np.zeros((32, 2, 3, 2, OC), dtype=np.float32)
    for c in range(64):
        khe, ic = divmod(c, IC)
        kt, p = divmod(c, 32)
        for dlt in range(2):
            kh = khe - dlt
            if 0 <= kh < KSZ:
                wl[p, kt, :, dlt, :] = conv_weight[:, ic, kh, :].T
    wl = wl.reshape(32, 2 * 384).astype(ml_dtypes.float8_e4m3)
    wts = np.concatenate([wl, wl], axis=0)  # rows 0-31 (img0) == 32-63 (img1)

    biasarr = np.tile(conv_bias.astype(np.float32), 2).reshape(128, 1)
    iden = np.eye(128, dtype=ml_dtypes.bfloat16)
    return xr, wts, biasarr, iden


def _make_in_maps(x, conv_weight, conv_bias):
    xr, wts, biasarr, iden = _host_prep(x, conv_weight, conv_bias)
    in_maps = []
    for c in range(N_CORES):
        xc = xr[c * B_LOC : (c + 1) * B_LOC]  # [16, 32, 2, FLAT]
        # pair p: img 2p on partitions 0-31, img 2p+1 on 32-63
        xc = np.ascontiguousarray(xc.reshape(PAIRS, 2, 32, 2 * FLAT)).reshape(
            PAIRS, 64, 2 * FLAT
        )
        in_maps.append(
            {"xrep": xc, "wts": wts, "bias": biasarr, "iden": iden}
        )
    return in_maps


def _assemble(res):
    y = np.concatenate([res.results[c]["y"] for c in range(N_CORES)], axis=0)
    # y[b, w', h'] -> out[b, h', w']
    return (
        np.ascontiguousarray(y.transpose(0, 2, 1))
        .reshape(B, 1, HO, WO)
        .astype(np.float32)
    )


def kernel(x, conv_weight, conv_bias):
    x = np.asarray(x, dtype=np.float32)
    conv_weight = np.asarray(conv_weight, dtype=np.float32)
    conv_bias = np.asarray(conv_bias, dtype=np.float32)

    in_maps = _make_in_maps(x, conv_weight, conv_bias)
    nc = _get_program()
    res = run_bass_kernel_spmd(nc, in_maps, list(range(N_CORES)))
    return _assemble(res)


# revision 11
# speedup vs baseline: 2.6516x; 2.6516x over previous
"""Trainium2 Bass kernel: conv2d(3x3, VALID) + bias -> channel-min -> tanh(tanh).

Full inputs in, full output out. Data-parallel over batch across 8 NeuronCores.

Scheme (fp8 weight-stationary conv + batched DMA transpose + 16-bit min):
  - Conv as matmul, weight-stationary: M packs (delta, oc) = 128 output
    partitions (delta = output-row parity via the khe = delta + kh trick),
    contraction K = (khe in [0,4), ic) = 64, moving tensor = 4 row-pairs x
    128 cols of the khe-shifted fp8 image (N = 512, kw enters as a free-dim
    offset).  fp8 halves the input DMA vs bf16 at the same PE rate.
  - Two images per pair run interleaved on PE row halves 0-63 / 64-127
    (tile_position row tiling) so stationary loads hide under streams.
  - PSUM f32 [128=(delta,oc), 512] is evacuated to SBUF bf16 with the conv
    bias fused (ScalarE Identity+bias, VectorE tensor_scalar for 1/4).
  - Four groups of evacuated tiles (16 t-rows) are DMA-xbar-transposed in
    ONE descriptor batch per quarter-image, alternating between the Pool
    and Sync HWDGE rings to spread descriptor-generation cost.
  - One wide 16-bit DVE tensor_reduce per quarter computes min over oc
    keeping (t, delta); double tanh on ScalarE; contiguous store as
    [img, w', h']; host transposes back.
"""

import os
import sys

for _p in ("/opt/trn_rl_repo", "/root/.axon_site/_ro/trn_rl_repo"):
    if os.path.isdir(_p) and _p not in sys.path:
        sys.path.insert(0, _p)

import numpy as np
import ml_dtypes

import concourse.bass as bass
import concourse.bacc as bacc
import concourse.tile as tile
from concourse import mybir
from concourse.bass_utils import run_bass_kernel_spmd

N_CORES = 8
B, IC, H, W = 128, 16, 128, 128
OC, KSZ = 64, 3
HO, WO = H - KSZ + 1, W - KSZ + 1  # 126, 126
B_LOC = B // N_CORES  # 16
PAIRS = B_LOC // 2  # 8
FLAT = H * W  # 16384
T = HO // 2  # 63 row-pairs per image (h' = 2t + delta)

BF16 = mybir.dt.bfloat16
FP8 = mybir.dt.float8e4
F32 = mybir.dt.float32
MIN = mybir.AluOpType.min

# quarters of 4 t-groups (16 t-rows; last quarter 15)
QUARTERS = []
for q4 in range(4):
    gs = []
    for gi in range(4):
        t0 = q4 * 16 + gi * 4
        gs.append((t0, min(4, T - t0)))
    QUARTERS.append(gs)


def _build_program():
    nc = bacc.Bacc(None)
    xr_hbm = nc.declare_dram_parameter(
        "xrep", [PAIRS, 128, FLAT], FP8, isOutput=False
    )
    w_hbm = nc.declare_dram_parameter("wts", [128, 3 * 128], FP8, isOutput=False)
    b_hbm = nc.declare_dram_parameter("bias", [128, 1], F32, isOutput=False)
    y_hbm = nc.declare_dram_parameter("y", [B_LOC, WO, HO], F32, isOutput=True)

    with tile.TileContext(nc) as tc:
        with (
            tc.tile_pool(name="const", bufs=1) as const,
            tc.tile_pool(name="xrp", bufs=2) as xrp,
            tc.tile_pool(name="psum", bufs=6, space="PSUM") as psump,
            tc.tile_pool(name="evac", bufs=4) as evacp,
            tc.tile_pool(name="tpsb", bufs=4) as tpsbp,
            tc.tile_pool(name="fin", bufs=4) as finp,
            tc.tile_pool(name="outp", bufs=4) as outp,
        ):
            w_sb = const.tile([128, 3 * 128], FP8)
            b_sb = const.tile([128, 1], F32)
            nc.sync.dma_start(w_sb[:], w_hbm[:])
            nc.sync.dma_start(b_sb[:], b_hbm[:])

            xr_tiles = {}

            def load_pair(p):
                xr_t = xrp.tile([128, FLAT], FP8, name="xr", tag="xr")
                nc.scalar.dma_start(xr_t[:], xr_hbm[p])
                xr_tiles[p] = xr_t

            load_pair(0)
            ucount = 0
            tcount = 0
            for pair in range(PAIRS):
                if pair + 1 < PAIRS:
                    load_pair(pair + 1)
                xr = xr_tiles.pop(pair)
                # [128, 64 row-pairs, 256]
                xrv = xr.rearrange("p (r q) -> p r q", q=2 * W)

                fins = [
                    finp.tile([128, HO], BF16, tag="fin", name="fin")
                    for _ in range(2)
                ]

                for q4, gs in enumerate(QUARTERS):
                    nt = sum(c for _, c in gs)  # 16 or 15
                    evs = [
                        evacp.tile([128, 16 * 128], BF16, tag="ev", name="ev")
                        for _ in range(2)
                    ]
                    for gi, (t0, cnt) in enumerate(gs):
                        n = cnt * 128
                        ps2 = [
                            psump.tile([128, 512], F32, name="ps", tag="ps")
                            for _ in range(2)
                        ]
                        for kw in range(3):
                            for h in range(2):
                                nc.tensor.matmul(
                                    ps2[h][:, :n],
                                    w_sb[
                                        64 * h : 64 * h + 64,
                                        kw * 128 : (kw + 1) * 128,
                                    ],
                                    xrv[
                                        64 * h : 64 * h + 64,
                                        t0 : t0 + cnt,
                                        kw : kw + 128,
                                    ],
                                    start=(kw == 0),
                                    stop=(kw == 2),
                                    tile_position=(64 * h, 0),
                                    skip_group_check=True,
                                )
                        for h in range(2):
                            # evacuate PSUM -> SBUF bf16 with fused bias
                            dst = evs[h][:, gi * 512 : gi * 512 + n]
                            ucount += 1
                            if ucount % 4 == 3:
                                nc.vector.tensor_scalar(
                                    dst, ps2[h][:, :n], b_sb[:, 0:1], None,
                                    mybir.AluOpType.add,
                                )
                            else:
                                nc.scalar.activation(
                                    dst, ps2[h][:, :n],
                                    mybir.ActivationFunctionType.Identity,
                                    bias=b_sb[:, 0:1],
                                )
                    for h in range(2):
                        # one batched xbar transpose per quarter-image:
                        # [128=(d,oc), nt*128=(t,w)] -> [128=w, (t,d,oc)]
                        tp = tpsbp.tile([128, 16 * 128], BF16, tag="tp", name="tp")
                        tpv3 = tp.rearrange("p (j c) -> p j c", c=128)
                        tcount += 1
                        eng = nc.scalar if tcount % 4 == 3 else nc.sync
                        eng.dma_start_transpose(
                            tpv3[:, :nt, :], evs[h][:, : nt * 128]
                        )
                        # min over oc keeping (t, delta)
                        tpv = tp.rearrange("p (j d c) -> p j d c", d=2, c=64)
                        nc.vector.tensor_reduce(
                            fins[h][:, q4 * 32 : q4 * 32 + 2 * nt],
                            tpv[:, :nt, :, :],
                            axis=mybir.AxisListType.X,
                            op=MIN,
                        )

                # double tanh + store  (partitions = w'; 126..127 garbage)
                for h in range(2):
                    th = outp.tile([128, HO], F32, tag="th", name="th")
                    ot = outp.tile([128, HO], F32, tag="ot", name="ot")
                    nc.scalar.activation(
                        th[0:WO, :], fins[h][0:WO, :],
                        mybir.ActivationFunctionType.Tanh,
                    )
                    nc.scalar.activation(
                        ot[0:WO, :], th[0:WO, :],
                        mybir.ActivationFunctionType.Tanh,
                    )
                    nc.scalar.dma_start(y_hbm[pair * 2 + h], ot[0:WO, :])
    nc.finalize()
    return nc


_NC_CACHE = None


def _get_program():
    global _NC_CACHE
    if _NC_CACHE is None:
        _NC_CACHE = _build_program()
    return _NC_CACHE


def _host_prep(x, conv_weight, conv_bias):
    # khe-shifted fp8 copies: xr[b, c=(khe,ic), r, w] = x[b, ic, r+khe, w]
    x8 = x.astype(ml_dtypes.float8_e4m3)
    xr = np.zeros((B, 4 * IC, H, W), dtype=ml_dtypes.float8_e4m3)
    for khe in range(4):
        xr[:, khe * IC : (khe + 1) * IC, : H - khe, :] = x8[:, :, khe:, :]
    xr = xr.reshape(B, 4 * IC, FLAT)

    # weights: wl[(khe,ic), kw, (delta,oc)] = w[oc, ic, khe-delta, kw]
    wl = np.zeros((64, 3, 128), dtype=np.float32)
    for khe in range(4):
        for dlt in range(2):
            kh = khe - dlt
            if 0 <= kh < KSZ:
                wl[khe * IC : khe * IC + IC, :, dlt * 64 : dlt * 64 + 64] = (
                    conv_weight[:, :, kh, :].transpose(1, 2, 0)
                )
    wts = np.concatenate([wl, wl], axis=0).reshape(128, 3 * 128)
    wts = wts.astype(ml_dtypes.float8_e4m3)

    biasarr = np.tile(conv_bias.astype(np.float32), 2).reshape(128, 1)
    return xr, wts, biasarr


def _make_in_maps(x, conv_weight, conv_bias):
    xr, wts, biasarr = _host_prep(x, conv_weight, conv_bias)
    in_maps = []
    for c in range(N_CORES):
        xc = xr[c * B_LOC : (c + 1) * B_LOC]  # [16, 64, FLAT]
        xc = np.ascontiguousarray(xc).reshape(PAIRS, 128, FLAT)
        in_maps.append({"xrep": xc, "wts": wts, "bias": biasarr})
    return in_maps


def _assemble(res):
    y = np.concatenate([res.results[c]["y"] for c in range(N_CORES)], axis=0)
    # y[b, w', h'] -> out[b, h', w']
    return (
        np.ascontiguousarray(y.transpose(0, 2, 1))
        .reshape(B, 1, HO, WO)
        .astype(np.float32)
    )


def kernel(x, conv_weight, conv_bias):
    x = np.asarray(x, dtype=np.float32)
    conv_weight = np.asarray(conv_weight, dtype=np.float32)
    conv_bias = np.asarray(conv_bias, dtype=np.float32)

    in_maps = _make_in_maps(x, conv_weight, conv_bias)
    nc = _get_program()
    res = run_bass_kernel_spmd(nc, in_maps, list(range(N_CORES)))
    return _assemble(res)


# revision 13
# speedup vs baseline: 2.7070x; 1.0209x over previous
"""Trainium2 Bass kernel: conv2d(3x3, VALID) + bias -> channel-min -> tanh(tanh).

Full inputs in, full output out. Data-parallel over batch across 8 NeuronCores.

Scheme (fp8 weight-stationary conv + batched DMA transpose + 16-bit min):
  - Conv as matmul, weight-stationary: M packs (delta, oc) = 128 output
    partitions (delta = output-row parity via the khe = delta + kh trick),
    contraction K = (khe in [0,4), ic) = 64, moving tensor = 4 row-pairs x
    128 cols of the khe-shifted fp8 image (N = 512, kw enters as a free-dim
    offset).  fp8 halves the input DMA vs bf16 at the same PE rate.
  - Two images per pair run interleaved on PE row halves 0-63 / 64-127
    (tile_position row tiling) so stationary loads hide under streams.
  - PSUM f32 [128=(delta,oc), 512] is evacuated to SBUF bf16 with the conv
    bias fused (ScalarE Identity+bias, VectorE tensor_scalar for 1/4).
  - Four groups of evacuated tiles (16 t-rows) are DMA-xbar-transposed in
    ONE descriptor batch per quarter-image, alternating between the Pool
    and Sync HWDGE rings to spread descriptor-generation cost.
  - One wide 16-bit DVE tensor_reduce per quarter computes min over oc
    keeping (t, delta); double tanh on ScalarE; contiguous store as
    [img, w', h']; host transposes back.
"""

import os
import sys

for _p in ("/opt/trn_rl_repo", "/root/.axon_site/_ro/trn_rl_repo"):
    if os.path.isdir(_p) and _p not in sys.path:
        sys.path.insert(0, _p)

import numpy as np
import ml_dtypes

import concourse.bass as bass
import concourse.bacc as bacc
import concourse.tile as tile
from concourse import mybir
from concourse.bass_utils import run_bass_kernel_spmd

N_CORES = 8
B, IC, H, W = 128, 16, 128, 128
OC, KSZ = 64, 3
HO, WO = H - KSZ + 1, W - KSZ + 1  # 126, 126
B_LOC = B // N_CORES  # 16
PAIRS = B_LOC // 2  # 8
FLAT = H * W  # 16384
T = HO // 2  # 63 row-pairs per image (h' = 2t + delta)

BF16 = mybir.dt.bfloat16
FP8 = mybir.dt.float8e4
F32 = mybir.dt.float32
MIN = mybir.AluOpType.min

# quarters of 4 t-groups (16 t-rows; last quarter 15)
QUARTERS = []
for q4 in range(4):
    gs = []
    for gi in range(4):
        t0 = q4 * 16 + gi * 4
        gs.append((t0, min(4, T - t0)))
    QUARTERS.append(gs)


def _build_program():
    nc = bacc.Bacc(None)
    xr_hbm = nc.declare_dram_parameter(
        "xrep", [PAIRS, 128, FLAT], FP8, isOutput=False
    )
    w_hbm = nc.declare_dram_parameter("wts", [128, 3 * 128], FP8, isOutput=False)
    b_hbm = nc.declare_dram_parameter("bias", [128, 1], F32, isOutput=False)
    y_hbm = nc.declare_dram_parameter("y", [B_LOC, WO, HO], F32, isOutput=True)

    with tile.TileContext(nc) as tc:
        with (
            tc.tile_pool(name="const", bufs=1) as const,
            tc.tile_pool(name="xrp", bufs=2) as xrp,
            tc.tile_pool(name="psum", bufs=6, space="PSUM") as psump,
            tc.tile_pool(name="evac", bufs=8) as evacp,
            tc.tile_pool(name="tpsb", bufs=6) as tpsbp,
            tc.tile_pool(name="fold", bufs=4) as foldp,
            tc.tile_pool(name="fin", bufs=4) as finp,
            tc.tile_pool(name="outp", bufs=4) as outp,
        ):
            w_sb = const.tile([128, 3 * 128], FP8)
            b_sb = const.tile([128, 1], F32)
            nc.sync.dma_start(w_sb[:], w_hbm[:])
            nc.sync.dma_start(b_sb[:], b_hbm[:])

            xr_tiles = {}

            def load_pair(p):
                xr_t = xrp.tile([128, FLAT], FP8, name="xr", tag="xr")
                nc.scalar.dma_start(xr_t[:], xr_hbm[p])
                xr_tiles[p] = xr_t

            load_pair(0)
            ucount = 0
            tcount = 0
            for pair in range(PAIRS):
                if pair + 1 < PAIRS:
                    load_pair(pair + 1)
                xr = xr_tiles.pop(pair)
                # [128, 64 row-pairs, 256]
                xrv = xr.rearrange("p (r q) -> p r q", q=2 * W)

                fins = [
                    finp.tile([128, HO], BF16, tag="fin", name="fin")
                    for _ in range(2)
                ]

                for q4, gs in enumerate(QUARTERS):
                    nt = sum(c for _, c in gs)  # 16 or 15
                    evs = [
                        evacp.tile([128, 16 * 128], BF16, tag="ev", name="ev")
                        for _ in range(2)
                    ]
                    for gi, (t0, cnt) in enumerate(gs):
                        n = cnt * 128
                        ps2 = [
                            psump.tile([128, 512], F32, name="ps", tag="ps")
                            for _ in range(2)
                        ]
                        for kw in range(3):
                            for h in range(2):
                                nc.tensor.matmul(
                                    ps2[h][:, :n],
                                    w_sb[
                                        64 * h : 64 * h + 64,
                                        kw * 128 : (kw + 1) * 128,
                                    ],
                                    xrv[
                                        64 * h : 64 * h + 64,
                                        t0 : t0 + cnt,
                                        kw : kw + 128,
                                    ],
                                    start=(kw == 0),
                                    stop=(kw == 2),
                                    tile_position=(64 * h, 0),
                                    skip_group_check=True,
                                )
                        for h in range(2):
                            # evacuate PSUM -> SBUF bf16 with fused bias
                            dst = evs[h][:, gi * 512 : gi * 512 + n]
                            ucount += 1
                            if ucount % 4 == 3:
                                nc.vector.tensor_scalar(
                                    dst, ps2[h][:, :n], b_sb[:, 0:1], None,
                                    mybir.AluOpType.add,
                                )
                            else:
                                nc.scalar.activation(
                                    dst, ps2[h][:, :n],
                                    mybir.ActivationFunctionType.Identity,
                                    bias=b_sb[:, 0:1],
                                )
                    for h in range(2):
                        # one batched xbar transpose per quarter-image:
                        # [128=(d,oc), nt*128=(t,w)] -> [128=w, (t,d,oc)]
                        tp = tpsbp.tile([128, 16 * 128], BF16, tag="tp", name="tp")
                        tpv3 = tp.rearrange("p (j c) -> p j c", c=128)
                        tcount += 1
                        eng = nc.scalar if tcount % 4 == 3 else nc.sync
                        eng.dma_start_transpose(
                            tpv3[:, :nt, :], evs[h][:, : nt * 128]
                        )
                        # min over oc keeping (t, delta): two pairwise bf16
                        # folds (DVE 16-bit fast path) + a final reduce
                        tpv = tp.rearrange("p (j d c) -> p j d c", d=2, c=64)
                        f1 = foldp.tile([128, 16 * 64], BF16, tag="f1", name="f1")
                        f1v = f1.rearrange("p (j d c) -> p j d c", d=2, c=32)
                        nc.vector.tensor_tensor(
                            f1v[:, :nt, :, :],
                            tpv[:, :nt, :, 0:32],
                            tpv[:, :nt, :, 32:64],
                            MIN,
                        )
                        f2 = foldp.tile([128, 16 * 32], BF16, tag="f2", name="f2")
                        f2v = f2.rearrange("p (j d c) -> p j d c", d=2, c=16)
                        nc.vector.tensor_tensor(
                            f2v[:, :nt, :, :],
                            f1v[:, :nt, :, 0:16],
                            f1v[:, :nt, :, 16:32],
                            MIN,
                        )
                        nc.vector.tensor_reduce(
                            fins[h][:, q4 * 32 : q4 * 32 + 2 * nt],
                            f2v[:, :nt, :, :],
                            axis=mybir.AxisListType.X,
                            op=MIN,
                        )

                # double tanh + store  (partitions = w'; 126..127 garbage)
                for h in range(2):
                    th = outp.tile([128, HO], F32, tag="th", name="th")
                    ot = outp.tile([128, HO], F32, tag="ot", name="ot")
                    nc.scalar.activation(
                        th[0:WO, :], fins[h][0:WO, :],
                        mybir.ActivationFunctionType.Tanh,
                    )
                    nc.scalar.activation(
                        ot[0:WO, :], th[0:WO, :],
                        mybir.ActivationFunctionType.Tanh,
                    )
                    nc.scalar.dma_start(y_hbm[pair * 2 + h], ot[0:WO, :])
    nc.finalize()
    return nc


_NC_CACHE = None


def _get_program():
    global _NC_CACHE
    if _NC_CACHE is None:
        _NC_CACHE = _build_program()
    return _NC_CACHE


def _host_prep(x, conv_weight, conv_bias):
    # khe-shifted fp8 copies: xr[b, c=(khe,ic), r, w] = x[b, ic, r+khe, w]
    x8 = x.astype(ml_dtypes.float8_e4m3)
    xr = np.zeros((B, 4 * IC, H, W), dtype=ml_dtypes.float8_e4m3)
    for khe in range(4):
        xr[:, khe * IC : (khe + 1) * IC, : H - khe, :] = x8[:, :, khe:, :]
    xr = xr.reshape(B, 4 * IC, FLAT)

    # weights: wl[(khe,ic), kw, (delta,oc)] = w[oc, ic, khe-delta, kw]
    wl = np.zeros((64, 3, 128), dtype=np.float32)
    for khe in range(4):
        for dlt in range(2):
            kh = khe - dlt
            if 0 <= kh < KSZ:
                wl[khe * IC : khe * IC + IC, :, dlt * 64 : dlt * 64 + 64] = (
                    conv_weight[:, :, kh, :].transpose(1, 2, 0)
                )
    wts = np.concatenate([wl, wl], axis=0).reshape(128, 3 * 128)
    wts = wts.astype(ml_dtypes.float8_e4m3)

    biasarr = np.tile(conv_bias.astype(np.float32), 2).reshape(128, 1)
    return xr, wts, biasarr


def _make_in_maps(x, conv_weight, conv_bias):
    xr, wts, biasarr = _host_prep(x, conv_weight, conv_bias)
    in_maps = []
    for c in range(N_CORES):
        xc = xr[c * B_LOC : (c + 1) * B_LOC]  # [16, 64, FLAT]
        xc = np.ascontiguousarray(xc).reshape(PAIRS, 128, FLAT)
        in_maps.append({"xrep": xc, "wts": wts, "bias": biasarr})
    return in_maps


def _assemble(res):
    y = np.concatenate([res.results[c]["y"] for c in range(N_CORES)], axis=0)
    # y[b, w', h'] -> out[b, h', w']
    return (
        np.ascontiguousarray(y.transpose(0, 2, 1))
        .reshape(B, 1, HO, WO)
        .astype(np.float32)
    )


def kernel(x, conv_weight, conv_bias):
    x = np.asarray(x, dtype=np.float32)
    conv_weight = np.asarray(conv_weight, dtype=np.float32)
    conv_bias = np.asarray(conv_bias, dtype=np.float32)

    in_maps = _make_in_maps(x, conv_weight, conv_bias)
    nc = _get_program()
    res = run_bass_kernel_spmd(nc, in_maps, list(range(N_CORES)))
    return _assemble(res)
